# revision 5
# baseline (speedup 1.0000x reference)
"""Trainium2 kernel for nn_EuclideanEmbedding (edge-scale + segment_sum).

Computes: out[n, :] = inv * sum_{e: receivers[e]==n} sh_vectors[e, :] * cutoffs[e]

Distribution strategy (host side, inside kernel()):
  - Edges are sharded across the 8 NeuronCores BY RECEIVER NODE RANGE:
    core c owns nodes [c*6250, (c+1)*6250) and receives exactly the edges
    targeting those nodes.  Each core produces its own disjoint slice of the
    output, so the final "all-reduce" degenerates to a concatenation.

Scatter-add on device uses gpsimd.dma_scatter_add (SWDGE descriptors with a
CCE add in the SDMA datapath: M2S reads src-SBUF + dst-HBM, adds, S2M writes
dst-HBM).  The read and write engines are decoupled, so two in-flight
descriptors hitting the SAME HBM address lose updates (verified on HW:
only deg>=2 nodes were wrong).  To make duplicates race-free the kernel:
  - keeps R=5 replica tables (dst row = r*6400 + local_node, int16-safe),
  - assigns occurrence o of a node to replica o%R and window o//R,
  - emits windows in order, node-sorted inside a window, so equal dst rows
    are >= GAP stream slots apart (asserted on host; in-flight window is
    bounded by the ~256-descriptor ring carveout ~= 4096 stream slots),
  - small windows are front-padded with zero-valued trash elements.
A final on-device pass sums the 5 replicas into the output slice.  The
inv_avg_num_neighbors factor is folded into the per-edge cutoff scale on
device.  ExternalOutput buffers are pre-zeroed by the runner (donated zero
buffers under PJRT / pre-zeroed out_maps on the native path).
"""

import math
import os

import numpy as np

# ---------------------------------------------------------------- constants
N_NODES = 50_000
D_SH = 16
N_CORES = 8
NPC = N_NODES // N_CORES          # 6250 nodes per core
R_REP = 5                         # replica tables (5*6400 = 32000 < 2^15)
ROWS_PER_REP = 6400               # 50 * 128
N_TAB = R_REP * ROWS_PER_REP      # 32000 table rows
TRASH_ROW = N_TAB - 1             # 31999: replica-4 row 6399, never read
STEP = 64                         # table row stride in f32 (= 256 bytes)
ELEM = D_SH                       # scatter payload per element (16 f32 = 64B)
NB = 32                           # 128-edge chunks per scatter instruction
                                  # (nb=64 -> 1025 tx descs/ring crashes the
                                  #  SWDGE ring carveout; nb<=32 verified)
IPB = NB * 128                    # scatter elements per block (4096)
GAP = 8192                        # min stream distance between equal dst rows
SAFE_MIN = 4096                   # hard assert threshold

_NC_CACHE: dict = {}
LAST_RESULTS = None  # BassKernelResults of the most recent run (for test.py)


# ---------------------------------------------------------------- device IR
def build_nc(n_blocks: int, nb: int = NB):
    """Build + compile the per-core Bass program."""
    key = (n_blocks, nb)
    if key in _NC_CACHE:
        return _NC_CACHE[key]

    import concourse.bacc as bacc
    import concourse.bass as bass
    import concourse.mybir as mybir
    from concourse import tile

    ipb = nb * 128
    nc = bacc.Bacc("TRN2", target_bir_lowering=False, debug=False)

    sh = nc.dram_tensor("sh", [n_blocks, 128, nb * D_SH], mybir.dt.float32,
                        kind="ExternalInput")
    cut = nc.dram_tensor("cut", [n_blocks, 128, nb], mybir.dt.float32,
                         kind="ExternalInput")
    idx = nc.dram_tensor("idx", [n_blocks, 128, ipb // 16], mybir.dt.int16,
                         kind="ExternalInput")
    inv = nc.dram_tensor("inv", [128, 1], mybir.dt.float32, kind="ExternalInput")
    tab = nc.dram_tensor("tab", [N_TAB, STEP], mybir.dt.float32,
                         kind="ExternalOutput")
    out = nc.dram_tensor("out", [ROWS_PER_REP, D_SH], mybir.dt.float32,
                         kind="ExternalOutput")

    with tile.TileContext(nc) as tc:
        with (
            tc.tile_pool(name="const", bufs=1) as cpool,
            tc.tile_pool(name="io", bufs=3) as pool,
            tc.tile_pool(name="red", bufs=2) as rpool,
        ):
            inv_t = cpool.tile([128, 1], mybir.dt.float32)
            nc.sync.dma_start(inv_t[:], inv[:])

            for b in range(n_blocks):
                sh_t = pool.tile([128, nb * D_SH], mybir.dt.float32, tag="sh")
                nc.sync.dma_start(sh_t[:], sh[b])
                cut_t = pool.tile([128, nb], mybir.dt.float32, tag="cut")
                nc.sync.dma_start(cut_t[:], cut[b])
                idx_t = pool.tile([128, ipb // 16], mybir.dt.int16, tag="idx")
                nc.sync.dma_start(idx_t[:], idx[b])

                # cut2 = cut * inv   (inv broadcast along free dim)
                cut2 = pool.tile([128, nb], mybir.dt.float32, tag="cut2")
                inv_b = bass.AP(inv_t[:].tensor, inv_t[:].offset,
                                [list(inv_t[:].ap[0]), [0, nb]])
                nc.vector.tensor_mul(cut2[:], cut_t[:], inv_b)

                # scaled[p, j, d] = sh[p, j, d] * cut2[p, j]
                scl = pool.tile([128, nb * D_SH], mybir.dt.float32, tag="scl")
                scl3 = scl[:].rearrange("p (n d) -> p n d", d=D_SH)
                sh3 = sh_t[:].rearrange("p (n d) -> p n d", d=D_SH)
                c2 = cut2[:]
                cut_b = bass.AP(c2.tensor, c2.offset,
                                [list(c2.ap[0]), list(c2.ap[1]), [0, D_SH]])
                nc.vector.tensor_mul(scl3, sh3, cut_b)

                # tab[idx_i, 0:16] += scaled element i
                nc.gpsimd.dma_scatter_add(
                    tab.ap()[:, 0:ELEM],
                    scl3,
                    idx_t[:],
                    ipb,
                    ipb,
                    ELEM,
                    elem_step=STEP,
                )

            # final: out[n, :] = sum_r tab[r*6400 + n, 0:16]
            # layout n = p*50 + j  ->  [128, 50, 16]
            acc = rpool.tile([128, 50 * D_SH], mybir.dt.float32, tag="acc")
            acc3 = acc[:].rearrange("p (j d) -> p j d", d=D_SH)
            reps = []
            for r in range(R_REP):
                tr = rpool.tile([128, 50 * D_SH], mybir.dt.float32,
                                tag=f"rep{r}")
                src = tab.ap()[r * ROWS_PER_REP:(r + 1) * ROWS_PER_REP, 0:D_SH]
                src3 = src.rearrange("(p j) d -> p j d", p=128)
                nc.sync.dma_start(tr[:].rearrange("p (j d) -> p j d", d=D_SH),
                                  src3)
                reps.append(tr)
            nc.vector.tensor_add(acc[:], reps[0][:], reps[1][:])
            nc.vector.tensor_add(acc[:], acc[:], reps[2][:])
            nc.vector.tensor_add(acc[:], acc[:], reps[3][:])
            nc.vector.tensor_add(acc[:], acc[:], reps[4][:])
            nc.sync.dma_start(out.ap().rearrange("(p j) d -> p j d", p=128),
                              acc3)

    nc.compile()
    _NC_CACHE[key] = nc
    return nc


# ---------------------------------------------------------------- host shard
def shard_inputs(sh_vectors, cutoffs, receivers, inv_avg_num_neighbors,
                 nb: int = NB, gap: int = GAP):
    """Partition + lay out edges for the 8 cores.  Returns (in_maps, n_blocks)."""
    sh_np = np.ascontiguousarray(np.asarray(sh_vectors, dtype=np.float32))
    cut_np = np.asarray(cutoffs, dtype=np.float32).ravel()
    rec = np.asarray(receivers).astype(np.int64).ravel()
    inv_val = np.float32(np.asarray(inv_avg_num_neighbors).ravel()[0])
    ipb = nb * 128

    order = np.argsort(rec, kind="stable")       # sorts by (core, local)
    rec_sorted = rec[order]
    bounds = np.searchsorted(rec_sorted, np.arange(0, N_NODES + 1, NPC))

    # per-core streams of (src edge id, dst row); -1 src = zero trash element
    core_src = []
    core_row = []
    for c in range(N_CORES):
        edges = order[bounds[c]:bounds[c + 1]]
        l = rec_sorted[bounds[c]:bounds[c + 1]] - c * NPC    # sorted local ids
        m = edges.shape[0]
        if m == 0:
            core_src.append(np.empty(0, np.int64))
            core_row.append(np.empty(0, np.int64))
            continue
        first = np.searchsorted(l, l, side="left")
        o = np.arange(m) - first                  # occurrence within node
        k = o // R_REP                            # window
        r = o - k * R_REP                         # replica
        row = r * ROWS_PER_REP + l
        perm = np.argsort(k, kind="stable")       # window-major, node-sorted
        k_s = k[perm]
        row_s = row[perm]
        src_s = edges[perm]
        # front-pad windows smaller than gap with trash elements
        win_sizes = np.bincount(k_s)
        pads = np.maximum(0, gap - win_sizes)
        pads[win_sizes == 0] = 0
        if pads.sum() == 0:
            core_src.append(src_s)
            core_row.append(row_s)
        else:
            seg_src = []
            seg_row = []
            w_starts = np.zeros(len(win_sizes) + 1, np.int64)
            w_starts[1:] = np.cumsum(win_sizes)
            for w in range(len(win_sizes)):
                if pads[w] > 0:
                    seg_src.append(np.full(pads[w], -1, np.int64))
                    seg_row.append(np.full(pads[w], TRASH_ROW, np.int64))
                seg_src.append(src_s[w_starts[w]:w_starts[w + 1]])
                seg_row.append(row_s[w_starts[w]:w_starts[w + 1]])
            core_src.append(np.concatenate(seg_src))
            core_row.append(np.concatenate(seg_row))

    max_len = max(s.shape[0] for s in core_src)
    n_blocks = max(1, math.ceil(max_len / ipb))
    e_pad = n_blocks * ipb

    in_maps = []
    inv_dev = np.full((128, 1), inv_val, dtype=np.float32)
    for c in range(N_CORES):
        m = core_src[c].shape[0]
        stream_src = np.full(e_pad, -1, dtype=np.int64)
        stream_row = np.full(e_pad, TRASH_ROW, dtype=np.int64)
        stream_src[:m] = core_src[c]
        stream_row[:m] = core_row[c]

        # safety: equal real dst rows must be >= SAFE_MIN apart in the stream
        real = stream_row != TRASH_ROW
        pos = np.nonzero(real)[0]
        rws = stream_row[pos]
        srt = np.lexsort((pos, rws))
        same = rws[srt][1:] == rws[srt][:-1]
        if same.any():
            dmin = np.diff(pos[srt])[same].min()
            thresh = min(SAFE_MIN, max(1, gap // 2))
            assert dmin >= thresh, (
                f"core {c}: duplicate dst rows only {dmin} apart (< {thresh})"
            )

        mask = stream_src >= 0
        src = stream_src[mask]
        sh_s = np.zeros((e_pad, D_SH), dtype=np.float32)
        sh_s[mask] = sh_np[src]
        cut_s = np.zeros(e_pad, dtype=np.float32)
        cut_s[mask] = cut_np[src]
        idx_s = stream_row.astype(np.int16)

        sh_dev = np.ascontiguousarray(
            sh_s.reshape(n_blocks, nb, 128, D_SH).transpose(0, 2, 1, 3)
            .reshape(n_blocks, 128, nb * D_SH)
        )
        cut_dev = np.ascontiguousarray(
            cut_s.reshape(n_blocks, nb, 128).transpose(0, 2, 1)
        )
        idx_dev = np.ascontiguousarray(
            idx_s.reshape(n_blocks, ipb // 16, 16).transpose(0, 2, 1)
        )
        idx_dev = np.ascontiguousarray(np.tile(idx_dev, (1, 8, 1)))

        in_maps.append({"sh": sh_dev, "cut": cut_dev, "idx": idx_dev,
                        "inv": inv_dev})
    return in_maps, n_blocks


# ---------------------------------------------------------------- profiling
def _install_ntff_shim() -> bool:
    """This image's antenv lacks the axon_hooks shim that bass_utils imports
    for trace=True under axon.  Recreate it from trn_agent_boot's ctypes hook
    so NTFF profiling works.  Returns True on success."""
    try:
        import sys
        import types

        import antenv

        if getattr(antenv, "axon_hooks", None) is not None:
            return True
        import trn_agent_boot.trn_boot as tb

        hook = tb._ntff_profile_via_ctypes("/opt/axon/libaxon_pjrt.so")
        mod = types.ModuleType("antenv.axon_hooks")
        mod._hook = hook
        mod.get_axon_ntff_profile_hook = lambda: mod._hook
        mod.set_axon_ntff_profile_hook = lambda h: setattr(mod, "_hook", h)
        sys.modules["antenv.axon_hooks"] = mod
        antenv.axon_hooks = mod
        return hook is not None
    except Exception as e:  # profiling is best-effort; the run must not break
        print(f"ntff shim unavailable: {e!r}")
        return False


# ---------------------------------------------------------------- entrypoint
def kernel(sh_vectors, cutoffs, receivers, inv_avg_num_neighbors) -> np.ndarray:
    global LAST_RESULTS
    from concourse.bass_utils import run_bass_kernel_spmd

    in_maps, n_blocks = shard_inputs(sh_vectors, cutoffs, receivers,
                                     inv_avg_num_neighbors)
    nc = build_nc(n_blocks)

    trace = os.environ.get("KERNEL_TRACE", "0") == "1"
    if trace:
        trace = _install_ntff_shim()
    res = run_bass_kernel_spmd(nc, in_maps, core_ids=list(range(N_CORES)),
                               trace=trace)
    LAST_RESULTS = res

    full = np.concatenate(
        [res.results[c]["out"][:NPC, :] for c in range(N_CORES)], axis=0
    )
    return np.ascontiguousarray(full.astype(np.float32, copy=False))


# revision 7
# speedup vs baseline: 22.3711x; 22.3711x over previous
"""Trainium2 kernel for nn_EuclideanEmbedding (edge-scale + segment_sum).

Computes: out[n, :] = inv * sum_{e: receivers[e]==n} sh_vectors[e, :] * cutoffs[e]

Distribution strategy (host side, inside kernel()):
  - Edges are sharded across the 8 NeuronCores BY RECEIVER NODE RANGE:
    core c owns nodes [c*6250, (c+1)*6250) and receives exactly the edges
    targeting those nodes.  Each core produces its own disjoint slice of the
    output, so the final "all-reduce" degenerates to a concatenation.
  - Within a core, edges are laid out DENSELY PER NODE: every (padded) node
    gets a fixed budget of CAP edge slots (CAP = max node degree rounded up,
    SPMD-uniform across cores); node n's edges occupy slots [0, deg_n), the
    rest are zero-filled.  This turns the segment_sum scatter into a purely
    dense, race-free segmented reduction — no scatter-add DMA at all.
    (A dma_scatter_add variant was measured at 3.85 ms: SWDGE descriptor
    generation is ~7.5 ns/element on the Pool engine and the per-element CCE
    read-add-write costs ~150 ns of DMA-ring time, so per-edge descriptors
    lose to reading ~43% zero padding by a wide margin.)

Device program per core (identical SPMD program, different data):
  preload cutoffs (node-slot layout) and inv; then for each of 50 node-tiles
  (128 nodes x CAP slots x 16): DMA the sh slots, DVE multiply by the
  broadcast cutoff, DVE tensor_reduce over the slot axis, multiply by inv,
  collect into an SBUF output tile; one DMA writes the [6400, 16] slice out.
Node n maps to partition n//50, column n%50 (partition-major within a tile).
"""

import math
import os

import numpy as np

# ---------------------------------------------------------------- constants
N_NODES = 50_000
D_SH = 16
N_CORES = 8
NPC = N_NODES // N_CORES          # 6250 nodes per core
NPAD = 6400                       # padded nodes per core (= 128 * 50)
JTILES = NPAD // 128              # 50 node-tiles per core

_NC_CACHE: dict = {}
LAST_RESULTS = None  # BassKernelResults of the most recent run (for test.py)


# ---------------------------------------------------------------- device IR
def build_nc(cap: int):
    """Build + compile the per-core Bass program for slot capacity `cap`."""
    if cap in _NC_CACHE:
        return _NC_CACHE[cap]

    import concourse.bacc as bacc
    import concourse.bass as bass
    import concourse.mybir as mybir
    from concourse import tile

    nc = bacc.Bacc("TRN2", target_bir_lowering=False, debug=False)

    # sh slots: tile j holds nodes n = p*JTILES + j  ->  [j, p, cap*16]
    sh = nc.dram_tensor("sh", [JTILES, 128, cap * D_SH], mybir.dt.float32,
                        kind="ExternalInput")
    cut = nc.dram_tensor("cut", [128, JTILES * cap], mybir.dt.float32,
                         kind="ExternalInput")
    inv = nc.dram_tensor("inv", [128, 1], mybir.dt.float32,
                         kind="ExternalInput")
    out = nc.dram_tensor("out", [NPAD, D_SH], mybir.dt.float32,
                         kind="ExternalOutput")

    with tile.TileContext(nc) as tc:
        with (
            tc.tile_pool(name="const", bufs=1) as cpool,
            tc.tile_pool(name="io", bufs=4) as pool,
        ):
            inv_t = cpool.tile([128, 1], mybir.dt.float32)
            nc.sync.dma_start(inv_t[:], inv[:])
            cut_t = cpool.tile([128, JTILES * cap], mybir.dt.float32)
            nc.sync.dma_start(cut_t[:], cut[:])
            out_sb = cpool.tile([128, JTILES * D_SH], mybir.dt.float32)
            out3 = out_sb[:].rearrange("p (j d) -> p j d", d=D_SH)

            inv_b = bass.AP(inv_t[:].tensor, inv_t[:].offset,
                            [list(inv_t[:].ap[0]), [0, D_SH]])

            for j in range(JTILES):
                sh_t = pool.tile([128, cap * D_SH], mybir.dt.float32, tag="sh")
                nc.sync.dma_start(sh_t[:], sh[j])

                # scl[p, s, d] = sh[p, s, d] * cut[p, j*cap + s]
                scl = pool.tile([128, cap * D_SH], mybir.dt.float32, tag="scl")
                scl3 = scl[:].rearrange("p (s d) -> p s d", d=D_SH)
                sh3 = sh_t[:].rearrange("p (s d) -> p s d", d=D_SH)
                cj = cut_t[:, j * cap:(j + 1) * cap]
                cut_b = bass.AP(cj.tensor, cj.offset,
                                [list(cj.ap[0]), list(cj.ap[1]), [0, D_SH]])
                nc.vector.tensor_mul(scl3, sh3, cut_b)

                # red[p, d] = sum_s scl[p, s, d]   (reduce innermost AP dim)
                red = pool.tile([128, D_SH], mybir.dt.float32, tag="red")
                scl_t = bass.AP(scl[:].tensor, scl[:].offset,
                                [list(scl[:].ap[0]), [1, D_SH], [D_SH, cap]])
                nc.vector.tensor_reduce(red[:], scl_t,
                                        mybir.AxisListType.X,
                                        mybir.AluOpType.add)

                # out_sb[:, j, :] = red * inv
                nc.vector.tensor_mul(out_sb[:, j * D_SH:(j + 1) * D_SH],
                                     red[:], inv_b)

            nc.sync.dma_start(out.ap().rearrange("(p j) d -> p j d", p=128),
                              out3)

    nc.compile()
    _NC_CACHE[cap] = nc
    return nc


# ---------------------------------------------------------------- host shard
def shard_inputs(sh_vectors, cutoffs, receivers, inv_avg_num_neighbors,
                 cap: int | None = None):
    """Partition edges by receiver range and build dense per-node slot
    layouts.  Returns (in_maps, cap)."""
    sh_np = np.ascontiguousarray(np.asarray(sh_vectors, dtype=np.float32))
    cut_np = np.asarray(cutoffs, dtype=np.float32).ravel()
    rec = np.asarray(receivers).astype(np.int64).ravel()
    inv_val = np.float32(np.asarray(inv_avg_num_neighbors).ravel()[0])

    order = np.argsort(rec, kind="stable")       # sorts by (core, local)
    rec_sorted = rec[order]
    # occurrence of each edge within its node
    first = np.searchsorted(rec_sorted, rec_sorted, side="left")
    occ = np.arange(rec.size) - first
    if cap is None:
        maxdeg = int(occ.max()) + 1 if rec.size else 1
        cap = max(8, math.ceil(maxdeg / 8) * 8)
    else:
        assert occ.size == 0 or int(occ.max()) < cap

    bounds = np.searchsorted(rec_sorted, np.arange(0, N_NODES + 1, NPC))

    in_maps = []
    inv_dev = np.full((128, 1), inv_val, dtype=np.float32)
    for c in range(N_CORES):
        lo, hi = bounds[c], bounds[c + 1]
        edges = order[lo:hi]
        l = rec_sorted[lo:hi] - c * NPC          # local node id, sorted
        o = occ[lo:hi]
        # node n -> tile j = n % JTILES, partition p = n // JTILES
        p = l // JTILES
        j = l - p * JTILES
        # sh_dev[j, p, (s, d)] ; flat slot index
        flat = (j * 128 + p) * cap + o
        sh_dev = np.zeros((JTILES * 128 * cap, D_SH), dtype=np.float32)
        sh_dev[flat] = sh_np[edges]
        sh_dev = sh_dev.reshape(JTILES, 128, cap * D_SH)
        # cut_dev[p, (j, s)]
        cut_dev = np.zeros((128, JTILES * cap), dtype=np.float32)
        cut_dev[p, j * cap + o] = cut_np[edges]
        in_maps.append({"sh": sh_dev, "cut": cut_dev, "inv": inv_dev})
    return in_maps, cap


# ---------------------------------------------------------------- profiling
def _install_ntff_shim() -> bool:
    """This image's antenv lacks the axon_hooks shim that bass_utils imports
    for trace=True under axon.  Recreate it from trn_agent_boot's ctypes hook
    so NTFF profiling works.  Returns True on success."""
    try:
        import sys
        import types

        import antenv

        if getattr(antenv, "axon_hooks", None) is not None:
            return True
        import trn_agent_boot.trn_boot as tb

        hook = tb._ntff_profile_via_ctypes("/opt/axon/libaxon_pjrt.so")
        mod = types.ModuleType("antenv.axon_hooks")
        mod._hook = hook
        mod.get_axon_ntff_profile_hook = lambda: mod._hook
        mod.set_axon_ntff_profile_hook = lambda h: setattr(mod, "_hook", h)
        sys.modules["antenv.axon_hooks"] = mod
        antenv.axon_hooks = mod
        return hook is not None
    except Exception as e:  # profiling is best-effort; the run must not break
        print(f"ntff shim unavailable: {e!r}")
        return False


# ---------------------------------------------------------------- entrypoint
def kernel(sh_vectors, cutoffs, receivers, inv_avg_num_neighbors) -> np.ndarray:
    global LAST_RESULTS
    from concourse.bass_utils import run_bass_kernel_spmd

    in_maps, cap = shard_inputs(sh_vectors, cutoffs, receivers,
                                inv_avg_num_neighbors)
    nc = build_nc(cap)

    trace = os.environ.get("KERNEL_TRACE", "0") == "1"
    if trace:
        trace = _install_ntff_shim()
    res = run_bass_kernel_spmd(nc, in_maps, core_ids=list(range(N_CORES)),
                               trace=trace)
    LAST_RESULTS = res

    full = np.concatenate(
        [res.results[c]["out"][:NPC, :] for c in range(N_CORES)], axis=0
    )
    return np.ascontiguousarray(full.astype(np.float32, copy=False))


# revision 9
# speedup vs baseline: 28.4628x; 1.2723x over previous
"""Trainium2 kernel for nn_EuclideanEmbedding (edge-scale + segment_sum).

Computes: out[n, :] = inv * sum_{e: receivers[e]==n} sh_vectors[e, :] * cutoffs[e]

Distribution strategy (host side, inside kernel()):
  - Edges are sharded across the 8 NeuronCores BY RECEIVER NODE RANGE:
    core c owns nodes [c*6250, (c+1)*6250) and receives exactly the edges
    targeting those nodes.  Each core produces its own disjoint slice of the
    output, so the final "all-reduce" degenerates to a concatenation.
  - Within a core, edges are laid out DENSELY PER NODE: every (padded) node
    gets a fixed budget of CAP edge slots (CAP = max node degree rounded up,
    SPMD-uniform across cores); node n's edges occupy slots [0, deg_n), the
    rest are zero-filled.  This turns the segment_sum scatter into a purely
    dense, race-free segmented reduction — no scatter-add DMA at all.
    (A dma_scatter_add variant was measured at 3.85 ms: SWDGE descriptor
    generation is ~7.5 ns/element on the Pool engine and the per-element CCE
    read-add-write costs ~150 ns of DMA-ring time, so per-edge descriptors
    lose to reading ~43% zero padding by a wide margin.)

Device program per core (identical SPMD program, different data):
  preload cutoffs (node-slot layout) and inv; then for each of 50 node-tiles
  (128 nodes x CAP slots x 16): DMA the sh slots, DVE multiply by the
  broadcast cutoff, DVE tensor_reduce over the slot axis, multiply by inv,
  collect into an SBUF output tile; one DMA writes the [6400, 16] slice out.
Node n maps to partition n//50, column n%50 (partition-major within a tile).
"""

import math
import os

import numpy as np

# ---------------------------------------------------------------- constants
N_NODES = 50_000
D_SH = 16
N_CORES = 8
NPC = N_NODES // N_CORES          # 6250 nodes per core
NPAD = 6400                       # padded nodes per core (= 128 * 50)
JTILES = NPAD // 128              # 50 node-tiles per core

_NC_CACHE: dict = {}
LAST_RESULTS = None  # BassKernelResults of the most recent run (for test.py)


# ---------------------------------------------------------------- device IR
JGRP = 5                          # node-tiles batched per DVE op / DMA
NGRP = JTILES // JGRP             # 10 groups


def build_nc(cap: int):
    """Build + compile the per-core Bass program for slot capacity `cap`."""
    if cap in _NC_CACHE:
        return _NC_CACHE[cap]

    import concourse.bacc as bacc
    import concourse.bass as bass
    import concourse.mybir as mybir
    from concourse import tile

    nc = bacc.Bacc("TRN2", target_bir_lowering=False, debug=False)

    # sh slots, d-major per node: group g, tile jj holds nodes
    # n = p*JTILES + (g*JGRP + jj)  ->  sh[g, p, jj, d, s]
    sh = nc.dram_tensor("sh", [NGRP, 128, JGRP * D_SH * cap],
                        mybir.dt.float32, kind="ExternalInput")
    cut = nc.dram_tensor("cut", [128, JTILES * cap], mybir.dt.float32,
                         kind="ExternalInput")
    inv = nc.dram_tensor("inv", [128, 1], mybir.dt.float32,
                         kind="ExternalInput")
    out = nc.dram_tensor("out", [NPAD, D_SH], mybir.dt.float32,
                         kind="ExternalOutput")

    gcols = JGRP * D_SH * cap

    with tile.TileContext(nc) as tc:
        with (
            tc.tile_pool(name="const", bufs=1) as cpool,
            tc.tile_pool(name="io", bufs=3) as pool,
        ):
            inv_t = cpool.tile([128, 1], mybir.dt.float32)
            nc.sync.dma_start(inv_t[:], inv[:])
            cut_t = cpool.tile([128, JTILES * cap], mybir.dt.float32)
            nc.sync.dma_start(cut_t[:], cut[:])
            out_sb = cpool.tile([128, JTILES * D_SH], mybir.dt.float32)

            # cut2 = cut * inv   (one op; inv broadcast over the free dim)
            cut2 = cpool.tile([128, JTILES * cap], mybir.dt.float32)
            inv_b = bass.AP(inv_t[:].tensor, inv_t[:].offset,
                            [list(inv_t[:].ap[0]), [0, JTILES * cap]])
            nc.vector.tensor_mul(cut2[:], cut_t[:], inv_b)

            for g in range(NGRP):
                sh_t = pool.tile([128, gcols], mybir.dt.float32, tag="sh")
                nc.sync.dma_start(sh_t[:], sh[g])

                pstride = sh_t[:].ap[0][0]
                # scl[p, jj, d, s] = sh[p, jj, d, s] * cut2[p, (g*JGRP+jj)*cap + s]
                scl = pool.tile([128, gcols], mybir.dt.float32, tag="scl")
                sh4 = bass.AP(sh_t[:].tensor, sh_t[:].offset,
                              [[pstride, 128], [D_SH * cap, JGRP],
                               [cap, D_SH], [1, cap]])
                scl4 = bass.AP(scl[:].tensor, scl[:].offset,
                               [[scl[:].ap[0][0], 128], [D_SH * cap, JGRP],
                                [cap, D_SH], [1, cap]])
                c2 = cut2[:, g * JGRP * cap:(g + 1) * JGRP * cap]
                cut_b = bass.AP(c2.tensor, c2.offset,
                                [list(c2.ap[0]), [cap, JGRP], [0, D_SH],
                                 [1, cap]])
                nc.vector.tensor_mul(scl4, sh4, cut_b)

                # out_sb[:, (g*JGRP+jj)*16 + d] = sum_s scl[p, jj, d, s]
                nc.vector.tensor_reduce(
                    out_sb[:, g * JGRP * D_SH:(g + 1) * JGRP * D_SH],
                    scl4, mybir.AxisListType.X, mybir.AluOpType.add)

            out3 = out_sb[:].rearrange("p (j d) -> p j d", d=D_SH)
            nc.sync.dma_start(out.ap().rearrange("(p j) d -> p j d", p=128),
                              out3)

    nc.compile()
    _NC_CACHE[cap] = nc
    return nc


# ---------------------------------------------------------------- host shard
def shard_inputs(sh_vectors, cutoffs, receivers, inv_avg_num_neighbors,
                 cap: int | None = None):
    """Partition edges by receiver range and build dense per-node slot
    layouts.  Returns (in_maps, cap)."""
    sh_np = np.ascontiguousarray(np.asarray(sh_vectors, dtype=np.float32))
    cut_np = np.asarray(cutoffs, dtype=np.float32).ravel()
    rec = np.asarray(receivers).astype(np.int64).ravel()
    inv_val = np.float32(np.asarray(inv_avg_num_neighbors).ravel()[0])

    order = np.argsort(rec, kind="stable")       # sorts by (core, local)
    rec_sorted = rec[order]
    # occurrence of each edge within its node
    first = np.searchsorted(rec_sorted, rec_sorted, side="left")
    occ = np.arange(rec.size) - first
    if cap is None:
        maxdeg = int(occ.max()) + 1 if rec.size else 1
        cap = max(8, math.ceil(maxdeg / 8) * 8)
    else:
        assert occ.size == 0 or int(occ.max()) < cap

    bounds = np.searchsorted(rec_sorted, np.arange(0, N_NODES + 1, NPC))

    in_maps = []
    inv_dev = np.full((128, 1), inv_val, dtype=np.float32)
    for c in range(N_CORES):
        lo, hi = bounds[c], bounds[c + 1]
        edges = order[lo:hi]
        l = rec_sorted[lo:hi] - c * NPC          # local node id, sorted
        o = occ[lo:hi]
        # node n -> tile j = n % JTILES, partition p = n // JTILES
        p = l // JTILES
        j = l - p * JTILES
        # sh_dev[j, p, d, s]  (d-major per node for a contiguous-s reduce)
        flat = j * 128 + p
        sh_dev = np.zeros((JTILES * 128, D_SH, cap), dtype=np.float32)
        sh_dev[flat, :, o] = sh_np[edges]
        # regroup to [NGRP, 128, JGRP, d, s] for batched DMAs
        sh_dev = np.ascontiguousarray(
            sh_dev.reshape(NGRP, JGRP, 128, D_SH, cap)
            .transpose(0, 2, 1, 3, 4)
            .reshape(NGRP, 128, JGRP * D_SH * cap)
        )
        # cut_dev[p, (j, s)]
        cut_dev = np.zeros((128, JTILES * cap), dtype=np.float32)
        cut_dev[p, j * cap + o] = cut_np[edges]
        in_maps.append({"sh": sh_dev, "cut": cut_dev, "inv": inv_dev})
    return in_maps, cap


# ---------------------------------------------------------------- profiling
def _install_ntff_shim() -> bool:
    """This image's antenv lacks the axon_hooks shim that bass_utils imports
    for trace=True under axon.  Recreate it from trn_agent_boot's ctypes hook
    so NTFF profiling works.  Returns True on success."""
    try:
        import sys
        import types

        import antenv

        if getattr(antenv, "axon_hooks", None) is not None:
            return True
        import trn_agent_boot.trn_boot as tb

        hook = tb._ntff_profile_via_ctypes("/opt/axon/libaxon_pjrt.so")
        mod = types.ModuleType("antenv.axon_hooks")
        mod._hook = hook
        mod.get_axon_ntff_profile_hook = lambda: mod._hook
        mod.set_axon_ntff_profile_hook = lambda h: setattr(mod, "_hook", h)
        sys.modules["antenv.axon_hooks"] = mod
        antenv.axon_hooks = mod
        return hook is not None
    except Exception as e:  # profiling is best-effort; the run must not break
        print(f"ntff shim unavailable: {e!r}")
        return False


# ---------------------------------------------------------------- entrypoint
def kernel(sh_vectors, cutoffs, receivers, inv_avg_num_neighbors) -> np.ndarray:
    global LAST_RESULTS
    from concourse.bass_utils import run_bass_kernel_spmd

    in_maps, cap = shard_inputs(sh_vectors, cutoffs, receivers,
                                inv_avg_num_neighbors)
    nc = build_nc(cap)

    trace = os.environ.get("KERNEL_TRACE", "0") == "1"
    if trace:
        trace = _install_ntff_shim()
    res = run_bass_kernel_spmd(nc, in_maps, core_ids=list(range(N_CORES)),
                               trace=trace)
    LAST_RESULTS = res

    full = np.concatenate(
        [res.results[c]["out"][:NPC, :] for c in range(N_CORES)], axis=0
    )
    return np.ascontiguousarray(full.astype(np.float32, copy=False))


# revision 10
# speedup vs baseline: 42.5297x; 1.4942x over previous
"""Trainium2 kernel for nn_EuclideanEmbedding (edge-scale + segment_sum).

Computes: out[n, :] = inv * sum_{e: receivers[e]==n} sh_vectors[e, :] * cutoffs[e]

Distribution strategy (host side, inside kernel()):
  - Edges are sharded across the 8 NeuronCores BY RECEIVER NODE RANGE:
    core c owns nodes [c*6250, (c+1)*6250) and receives exactly the edges
    targeting those nodes.  Each core produces its own disjoint slice of the
    output, so the final "all-reduce" degenerates to a concatenation.
  - Within a core, edges are laid out DENSELY PER NODE: nodes are ordered by
    degree (descending) and split into 10 groups of 640; every node in group
    g gets a fixed budget of cap_g slots (max degree in the group, rounded
    up, SPMD-uniform across cores).  Node q's edges occupy slots [0, deg),
    the rest are zero-filled.  This turns the segment_sum scatter into a
    purely dense, race-free segmented reduction — no scatter-add DMA at all
    — while the degree sort keeps zero-padding small.
    (A dma_scatter_add variant was measured at 3.85 ms: SWDGE descriptor
    generation is ~7.5 ns/element on the Pool engine and the per-element CCE
    read-add-write costs ~150 ns of DMA-ring time.  The dense reduction with
    flat capacity measured 135 us; degree-sorted capacity cuts the padded
    volume further.)

Device program per core (identical SPMD program, different data):
  preload cutoffs (node-slot layout) and inv, fold inv into the cutoffs once;
  then per group: DMA the [128, 5*16*cap_g] sh slots (d-major per node so the
  slot reduction is contiguous), multiply by the broadcast cutoff (split
  between the Vector and GpSimd engines to balance load), tensor_reduce over
  the slot axis straight into the output tile; one DMA writes [6400, 16] out.
Sorted node position q maps to tile j = q // 128 (group g = j // 5),
partition p = q % 128; the host inverts the degree-sort permutation while
assembling the full output.
"""

import os

import numpy as np

# ---------------------------------------------------------------- constants
N_NODES = 50_000
D_SH = 16
N_CORES = 8
NPC = N_NODES // N_CORES          # 6250 nodes per core
NPAD = 6400                       # padded nodes per core
JTILES = NPAD // 128              # 50 node-tiles per core
JGRP = 5                          # node-tiles per group (per DVE op / DMA)
NGRP = JTILES // JGRP             # 10 groups of 640 nodes
CAP_Q = 4                         # capacity quantum
# groups whose multiply runs on GpSimd instead of Vector (load balancing;
# Vector also does all the reductions)
GP_MUL_GROUPS = frozenset({1, 3, 5, 7, 8, 9})

_NC_CACHE: dict = {}
LAST_RESULTS = None  # BassKernelResults of the most recent run (for test.py)


# ---------------------------------------------------------------- device IR
def build_nc(caps: tuple):
    """Build + compile the per-core Bass program for per-group slot
    capacities `caps` (len NGRP)."""
    key = tuple(caps)
    if key in _NC_CACHE:
        return _NC_CACHE[key]

    import concourse.bacc as bacc
    import concourse.bass as bass
    import concourse.mybir as mybir
    from concourse import tile

    nc = bacc.Bacc("TRN2", target_bir_lowering=False, debug=False)

    gcols = [JGRP * D_SH * c for c in caps]      # f32 per partition per group
    goffs = np.concatenate([[0], np.cumsum([128 * gc for gc in gcols])])
    cutcols = [JGRP * c for c in caps]
    cutoffs_off = np.concatenate([[0], np.cumsum(cutcols)])
    tot_cut = int(cutoffs_off[-1])

    sh = nc.dram_tensor("sh", [int(goffs[-1])], mybir.dt.float32,
                        kind="ExternalInput")
    cut = nc.dram_tensor("cut", [128, tot_cut], mybir.dt.float32,
                         kind="ExternalInput")
    inv = nc.dram_tensor("inv", [128, 1], mybir.dt.float32,
                         kind="ExternalInput")
    out = nc.dram_tensor("out", [NPAD, D_SH], mybir.dt.float32,
                         kind="ExternalOutput")

    with tile.TileContext(nc) as tc:
        with (
            tc.tile_pool(name="const", bufs=1) as cpool,
            tc.tile_pool(name="io", bufs=3) as pool,
        ):
            inv_t = cpool.tile([128, 1], mybir.dt.float32)
            nc.sync.dma_start(inv_t[:], inv[:])
            cut_t = cpool.tile([128, tot_cut], mybir.dt.float32)
            nc.sync.dma_start(cut_t[:], cut[:])
            out_sb = cpool.tile([128, JTILES * D_SH], mybir.dt.float32)

            # cut2 = cut * inv   (one op; inv broadcast over the free dim)
            cut2 = cpool.tile([128, tot_cut], mybir.dt.float32)
            inv_b = bass.AP(inv_t[:].tensor, inv_t[:].offset,
                            [list(inv_t[:].ap[0]), [0, tot_cut]])
            nc.vector.tensor_mul(cut2[:], cut_t[:], inv_b)

            for g in range(NGRP):
                cap = caps[g]
                gc = gcols[g]
                sh_t = pool.tile([128, gc], mybir.dt.float32, tag="sh")
                src = bass.AP(sh.ap().tensor, int(goffs[g]),
                              [[gc, 128], [1, gc]])
                nc.sync.dma_start(sh_t[:], src)

                # scl[p, jj, d, s] = sh[p, jj, d, s] * cut2[p, goff + jj*cap + s]
                scl = pool.tile([128, gc], mybir.dt.float32, tag="scl")
                sh4 = bass.AP(sh_t[:].tensor, sh_t[:].offset,
                              [list(sh_t[:].ap[0]), [D_SH * cap, JGRP],
                               [cap, D_SH], [1, cap]])
                scl4 = bass.AP(scl[:].tensor, scl[:].offset,
                               [list(scl[:].ap[0]), [D_SH * cap, JGRP],
                                [cap, D_SH], [1, cap]])
                c2 = cut2[:, int(cutoffs_off[g]):int(cutoffs_off[g + 1])]
                cut_b = bass.AP(c2.tensor, c2.offset,
                                [list(c2.ap[0]), [cap, JGRP], [0, D_SH],
                                 [1, cap]])
                if g in GP_MUL_GROUPS:
                    nc.gpsimd.tensor_mul(scl4, sh4, cut_b)
                else:
                    nc.vector.tensor_mul(scl4, sh4, cut_b)

                # out_sb[:, (g*JGRP+jj)*16 + d] = sum_s scl[p, jj, d, s]
                nc.vector.tensor_reduce(
                    out_sb[:, g * JGRP * D_SH:(g + 1) * JGRP * D_SH],
                    scl4, mybir.AxisListType.X, mybir.AluOpType.add)

            out3 = out_sb[:].rearrange("p (j d) -> p j d", d=D_SH)
            nc.sync.dma_start(out.ap().rearrange("(j p) d -> p j d", p=128),
                              out3)

    nc.compile()
    _NC_CACHE[key] = nc
    return nc


# ---------------------------------------------------------------- host shard
def shard_inputs(sh_vectors, cutoffs, receivers, inv_avg_num_neighbors):
    """Partition edges by receiver range, degree-sort nodes, build dense
    per-node slot layouts.  Returns (in_maps, caps, node_orders)."""
    sh_np = np.ascontiguousarray(np.asarray(sh_vectors, dtype=np.float32))
    cut_np = np.asarray(cutoffs, dtype=np.float32).ravel()
    rec = np.asarray(receivers).astype(np.int64).ravel()
    inv_val = np.float32(np.asarray(inv_avg_num_neighbors).ravel()[0])

    order = np.argsort(rec, kind="stable")       # sorts by (core, local)
    rec_sorted = rec[order]
    first = np.searchsorted(rec_sorted, rec_sorted, side="left")
    occ = np.arange(rec.size) - first            # occurrence within node
    bounds = np.searchsorted(rec_sorted, np.arange(0, N_NODES + 1, NPC))

    # per-core degree tables and degree-sorted node orders
    degs = np.zeros((N_CORES, NPAD), dtype=np.int64)
    node_orders = []
    pos_of_node = []
    for c in range(N_CORES):
        lseg = rec_sorted[bounds[c]:bounds[c + 1]] - c * NPC
        d = np.bincount(lseg, minlength=NPAD)
        degs[c] = d
        no = np.argsort(-d, kind="stable")       # position q -> node id
        node_orders.append(no)
        pon = np.empty(NPAD, dtype=np.int64)
        pon[no] = np.arange(NPAD)
        pos_of_node.append(pon)

    # per-group capacities: max degree among positions [g*640, (g+1)*640),
    # maximized across cores, rounded up to CAP_Q
    caps = []
    for g in range(NGRP):
        mx = 1
        for c in range(N_CORES):
            seg = degs[c][node_orders[c][g * 640:(g + 1) * 640]]
            if seg.size:
                mx = max(mx, int(seg.max()))
        caps.append(int(-(-mx // CAP_Q) * CAP_Q))
    caps = tuple(caps)

    gcols = [JGRP * D_SH * cp for cp in caps]
    goffs = np.concatenate([[0], np.cumsum([128 * gc for gc in gcols])])
    cutcols = [JGRP * cp for cp in caps]
    cutoffs_off = np.concatenate([[0], np.cumsum(cutcols)])
    tot_cut = int(cutoffs_off[-1])

    # per-(group) base offset helpers for a node position q:
    #   g = q // 640, j = q // 128, p = q % 128, jj = j - g*JGRP
    #   sh flat elem = goffs[g] + p*gcols[g] + jj*(16*cap) + d*cap + s
    #   cut col      = cutoffs_off[g] + jj*cap + s   (row p)
    in_maps = []
    inv_dev = np.full((128, 1), inv_val, dtype=np.float32)
    cap_arr = np.asarray(caps, dtype=np.int64)
    goffs_a = goffs.astype(np.int64)
    cutoffs_a = cutoffs_off.astype(np.int64)
    for c in range(N_CORES):
        lo, hi = bounds[c], bounds[c + 1]
        edges = order[lo:hi]
        l = rec_sorted[lo:hi] - c * NPC          # local node id, sorted
        o = occ[lo:hi]
        q = pos_of_node[c][l]                    # degree-sorted position
        g = q // 640
        j = q // 128
        p = q - j * 128
        jj = j - g * JGRP
        cap_e = cap_arr[g]
        flat = (goffs_a[g] + p * (JGRP * D_SH * cap_e)
                + jj * (D_SH * cap_e) + o)       # d=0 element; d stride = cap
        sh_dev = np.zeros(int(goffs_a[-1]), dtype=np.float32)
        # write all 16 d-components with stride cap_e
        base = flat
        shv = sh_np[edges]
        for d in range(D_SH):
            sh_dev[base + d * cap_e] = shv[:, d]
        cut_dev = np.zeros((128, tot_cut), dtype=np.float32)
        cut_dev[p, cutoffs_a[g] + jj * cap_e + o] = cut_np[edges]
        in_maps.append({"sh": sh_dev, "cut": cut_dev, "inv": inv_dev})
    return in_maps, caps, node_orders


# ---------------------------------------------------------------- profiling
def _install_ntff_shim() -> bool:
    """This image's antenv lacks the axon_hooks shim that bass_utils imports
    for trace=True under axon.  Recreate it from trn_agent_boot's ctypes hook
    so NTFF profiling works.  Returns True on success."""
    try:
        import sys
        import types

        import antenv

        if getattr(antenv, "axon_hooks", None) is not None:
            return True
        import trn_agent_boot.trn_boot as tb

        hook = tb._ntff_profile_via_ctypes("/opt/axon/libaxon_pjrt.so")
        mod = types.ModuleType("antenv.axon_hooks")
        mod._hook = hook
        mod.get_axon_ntff_profile_hook = lambda: mod._hook
        mod.set_axon_ntff_profile_hook = lambda h: setattr(mod, "_hook", h)
        sys.modules["antenv.axon_hooks"] = mod
        antenv.axon_hooks = mod
        return hook is not None
    except Exception as e:  # profiling is best-effort; the run must not break
        print(f"ntff shim unavailable: {e!r}")
        return False


# ---------------------------------------------------------------- entrypoint
def kernel(sh_vectors, cutoffs, receivers, inv_avg_num_neighbors) -> np.ndarray:
    global LAST_RESULTS
    from concourse.bass_utils import run_bass_kernel_spmd

    in_maps, caps, node_orders = shard_inputs(sh_vectors, cutoffs, receivers,
                                              inv_avg_num_neighbors)
    nc = build_nc(caps)

    trace = os.environ.get("KERNEL_TRACE", "0") == "1"
    if trace:
        trace = _install_ntff_shim()
    res = run_bass_kernel_spmd(nc, in_maps, core_ids=list(range(N_CORES)),
                               trace=trace)
    LAST_RESULTS = res

    full = np.empty((N_NODES, D_SH), dtype=np.float32)
    for c in range(N_CORES):
        o = res.results[c]["out"]                # row q -> node node_orders[q]
        blk = np.empty((NPAD, D_SH), dtype=np.float32)
        blk[node_orders[c]] = o
        full[c * NPC:(c + 1) * NPC] = blk[:NPC]
    return full


# revision 14
# speedup vs baseline: 47.2033x; 1.1099x over previous
"""Trainium2 kernel for nn_EuclideanEmbedding (edge-scale + segment_sum).

Computes: out[n, :] = inv * sum_{e: receivers[e]==n} sh_vectors[e, :] * cutoffs[e]

Distribution strategy (host side, inside kernel()):
  - Edges are sharded across the 8 NeuronCores BY RECEIVER NODE RANGE:
    core c owns nodes [c*6250, (c+1)*6250) and receives exactly the edges
    targeting those nodes.  Each core produces its own disjoint slice of the
    output, so the final "all-reduce" degenerates to a concatenation.
  - Within a core, edges are laid out DENSELY PER NODE: nodes are ordered by
    degree (descending) and split into 10 groups of 640; every node in group
    g gets a fixed budget of cap_g slots (max degree in the group, rounded
    up, SPMD-uniform across cores).  Node q's edges occupy slots [0, deg),
    the rest are zero-filled.  This turns the segment_sum scatter into a
    purely dense, race-free segmented reduction — no scatter-add DMA at all
    — while the degree sort keeps zero-padding small.
    (A dma_scatter_add variant was measured at 3.85 ms: SWDGE descriptor
    generation is ~7.5 ns/element on the Pool engine and the per-element CCE
    read-add-write costs ~150 ns of DMA-ring time.  The dense reduction with
    flat capacity measured 135 us; degree-sorted capacity cuts the padded
    volume further.)

Device program per core (identical SPMD program, different data):
  preload cutoffs (node-slot layout) and inv, fold inv into the cutoffs once;
  then per group: DMA the [128, 5*16*cap_g] sh slots (d-major per node so the
  slot reduction is contiguous), multiply by the broadcast cutoff (split
  between the Vector and GpSimd engines to balance load), tensor_reduce over
  the slot axis straight into the output tile; one DMA writes [6400, 16] out.
Sorted node position q maps to tile j = q // 128 (group g = j // 5),
partition p = q % 128; the host inverts the degree-sort permutation while
assembling the full output.
"""

import os

import numpy as np

# ---------------------------------------------------------------- constants
N_NODES = 50_000
D_SH = 16
N_CORES = 8
NPC = N_NODES // N_CORES          # 6250 nodes per core
NPAD = 6400                       # padded nodes per core
JTILES = NPAD // 128              # 50 node-tiles per core
JGRP = 5                          # node-tiles per group (per DVE op / DMA)
NGRP = JTILES // JGRP             # 10 groups of 640 nodes
CAP_Q = 4                         # capacity quantum
# groups whose multiply runs on GpSimd instead of Vector (load balancing;
# Vector also does all the reductions)
GP_MUL_GROUPS = frozenset({1, 3, 4, 5, 7, 8, 9})

_NC_CACHE: dict = {}
LAST_RESULTS = None  # BassKernelResults of the most recent run (for test.py)


# ---------------------------------------------------------------- device IR
def build_nc(caps: tuple):
    """Build + compile the per-core Bass program for per-group slot
    capacities `caps` (len NGRP)."""
    key = tuple(caps)
    if key in _NC_CACHE:
        return _NC_CACHE[key]

    import concourse.bacc as bacc
    import concourse.bass as bass
    import concourse.mybir as mybir
    from concourse import tile

    nc = bacc.Bacc("TRN2", target_bir_lowering=False, debug=False)

    gcols = [JGRP * D_SH * c for c in caps]      # f32 per partition per group
    goffs = np.concatenate([[0], np.cumsum([128 * gc for gc in gcols])])
    cutcols = [JGRP * c for c in caps]
    cutoffs_off = np.concatenate([[0], np.cumsum(cutcols)])
    tot_cut = int(cutoffs_off[-1])

    sh = nc.dram_tensor("sh", [int(goffs[-1])], mybir.dt.float32,
                        kind="ExternalInput")
    cut = nc.dram_tensor("cut", [128, tot_cut], mybir.dt.float32,
                         kind="ExternalInput")
    inv = nc.dram_tensor("inv", [128, 1], mybir.dt.float32,
                         kind="ExternalInput")
    out = nc.dram_tensor("out", [NPAD, D_SH], mybir.dt.float32,
                         kind="ExternalOutput")

    with tile.TileContext(nc) as tc:
        with (
            tc.tile_pool(name="const", bufs=1) as cpool,
            tc.tile_pool(name="io", bufs=4) as pool,
        ):
            inv_t = cpool.tile([128, 1], mybir.dt.float32)
            nc.sync.dma_start(inv_t[:], inv[:])
            cut_t = cpool.tile([128, tot_cut], mybir.dt.float32)
            nc.sync.dma_start(cut_t[:], cut[:])
            out_sb = cpool.tile([128, JTILES * D_SH], mybir.dt.float32)

            # cut2 = cut * inv   (one op; inv broadcast over the free dim)
            cut2 = cpool.tile([128, tot_cut], mybir.dt.float32)
            inv_b = bass.AP(inv_t[:].tensor, inv_t[:].offset,
                            [list(inv_t[:].ap[0]), [0, tot_cut]])
            nc.vector.tensor_mul(cut2[:], cut_t[:], inv_b)

            for g in range(NGRP):
                cap = caps[g]
                gc = gcols[g]
                sh_t = pool.tile([128, gc], mybir.dt.float32, tag="sh")
                src = bass.AP(sh.ap().tensor, int(goffs[g]),
                              [[gc, 128], [1, gc]])
                nc.sync.dma_start(sh_t[:], src)

                # scl[p, jj, d, s] = sh[p, jj, d, s] * cut2[p, goff + jj*cap + s]
                scl = pool.tile([128, gc], mybir.dt.float32, tag="scl")
                sh4 = bass.AP(sh_t[:].tensor, sh_t[:].offset,
                              [list(sh_t[:].ap[0]), [D_SH * cap, JGRP],
                               [cap, D_SH], [1, cap]])
                scl4 = bass.AP(scl[:].tensor, scl[:].offset,
                               [list(scl[:].ap[0]), [D_SH * cap, JGRP],
                                [cap, D_SH], [1, cap]])
                c2 = cut2[:, int(cutoffs_off[g]):int(cutoffs_off[g + 1])]
                cut_b = bass.AP(c2.tensor, c2.offset,
                                [list(c2.ap[0]), [cap, JGRP], [0, D_SH],
                                 [1, cap]])
                if g in GP_MUL_GROUPS:
                    nc.gpsimd.tensor_mul(scl4, sh4, cut_b)
                else:
                    nc.vector.tensor_mul(scl4, sh4, cut_b)

                # red[p, (jj, d)] = sum_s scl[p, jj, d, s]
                red = pool.tile([128, JGRP * D_SH], mybir.dt.float32,
                                tag="red")
                nc.vector.tensor_reduce(red[:], scl4, mybir.AxisListType.X,
                                        mybir.AluOpType.add)
                # collect on the (idle) scalar engine to avoid cross-group
                # serialization on a shared reduce target
                nc.scalar.mul(out_sb[:, g * JGRP * D_SH:(g + 1) * JGRP * D_SH],
                              red[:], 1.0)

            out3 = out_sb[:].rearrange("p (j d) -> p j d", d=D_SH)
            nc.sync.dma_start(out.ap().rearrange("(j p) d -> p j d", p=128),
                              out3)

    nc.compile()
    _NC_CACHE[key] = nc
    return nc


# ---------------------------------------------------------------- host shard
def shard_inputs(sh_vectors, cutoffs, receivers, inv_avg_num_neighbors):
    """Partition edges by receiver range, degree-sort nodes, build dense
    per-node slot layouts.  Returns (in_maps, caps, node_orders)."""
    sh_np = np.ascontiguousarray(np.asarray(sh_vectors, dtype=np.float32))
    cut_np = np.asarray(cutoffs, dtype=np.float32).ravel()
    rec = np.asarray(receivers).astype(np.int64).ravel()
    inv_val = np.float32(np.asarray(inv_avg_num_neighbors).ravel()[0])

    order = np.argsort(rec, kind="stable")       # sorts by (core, local)
    rec_sorted = rec[order]
    first = np.searchsorted(rec_sorted, rec_sorted, side="left")
    occ = np.arange(rec.size) - first            # occurrence within node
    bounds = np.searchsorted(rec_sorted, np.arange(0, N_NODES + 1, NPC))

    # per-core degree tables and degree-sorted node orders
    degs = np.zeros((N_CORES, NPAD), dtype=np.int64)
    node_orders = []
    pos_of_node = []
    for c in range(N_CORES):
        lseg = rec_sorted[bounds[c]:bounds[c + 1]] - c * NPC
        d = np.bincount(lseg, minlength=NPAD)
        degs[c] = d
        no = np.argsort(-d, kind="stable")       # position q -> node id
        node_orders.append(no)
        pon = np.empty(NPAD, dtype=np.int64)
        pon[no] = np.arange(NPAD)
        pos_of_node.append(pon)

    # per-group capacities: max degree among positions [g*640, (g+1)*640),
    # maximized across cores, rounded up to CAP_Q
    caps = []
    for g in range(NGRP):
        mx = 1
        for c in range(N_CORES):
            seg = degs[c][node_orders[c][g * 640:(g + 1) * 640]]
            if seg.size:
                mx = max(mx, int(seg.max()))
        caps.append(int(-(-mx // CAP_Q) * CAP_Q))
    caps = tuple(caps)

    gcols = [JGRP * D_SH * cp for cp in caps]
    goffs = np.concatenate([[0], np.cumsum([128 * gc for gc in gcols])])
    cutcols = [JGRP * cp for cp in caps]
    cutoffs_off = np.concatenate([[0], np.cumsum(cutcols)])
    tot_cut = int(cutoffs_off[-1])

    # per-(group) base offset helpers for a node position q:
    #   g = q // 640, j = q // 128, p = q % 128, jj = j - g*JGRP
    #   sh flat elem = goffs[g] + p*gcols[g] + jj*(16*cap) + d*cap + s
    #   cut col      = cutoffs_off[g] + jj*cap + s   (row p)
    in_maps = []
    inv_dev = np.full((128, 1), inv_val, dtype=np.float32)
    cap_arr = np.asarray(caps, dtype=np.int64)
    goffs_a = goffs.astype(np.int64)
    cutoffs_a = cutoffs_off.astype(np.int64)
    for c in range(N_CORES):
        lo, hi = bounds[c], bounds[c + 1]
        edges = order[lo:hi]
        l = rec_sorted[lo:hi] - c * NPC          # local node id, sorted
        o = occ[lo:hi]
        q = pos_of_node[c][l]                    # degree-sorted position
        g = q // 640
        j = q // 128
        p = q - j * 128
        jj = j - g * JGRP
        cap_e = cap_arr[g]
        flat = (goffs_a[g] + p * (JGRP * D_SH * cap_e)
                + jj * (D_SH * cap_e) + o)       # d=0 element; d stride = cap
        sh_dev = np.zeros(int(goffs_a[-1]), dtype=np.float32)
        # write all 16 d-components with stride cap_e
        base = flat
        shv = sh_np[edges]
        for d in range(D_SH):
            sh_dev[base + d * cap_e] = shv[:, d]
        cut_dev = np.zeros((128, tot_cut), dtype=np.float32)
        cut_dev[p, cutoffs_a[g] + jj * cap_e + o] = cut_np[edges]
        in_maps.append({"sh": sh_dev, "cut": cut_dev, "inv": inv_dev})
    return in_maps, caps, node_orders


# ---------------------------------------------------------------- profiling
def _install_ntff_shim() -> bool:
    """This image's antenv lacks the axon_hooks shim that bass_utils imports
    for trace=True under axon.  Recreate it from trn_agent_boot's ctypes hook
    so NTFF profiling works.  Returns True on success."""
    try:
        import sys
        import types

        import antenv

        if getattr(antenv, "axon_hooks", None) is not None:
            return True
        import trn_agent_boot.trn_boot as tb

        hook = tb._ntff_profile_via_ctypes("/opt/axon/libaxon_pjrt.so")
        mod = types.ModuleType("antenv.axon_hooks")
        mod._hook = hook
        mod.get_axon_ntff_profile_hook = lambda: mod._hook
        mod.set_axon_ntff_profile_hook = lambda h: setattr(mod, "_hook", h)
        sys.modules["antenv.axon_hooks"] = mod
        antenv.axon_hooks = mod
        return hook is not None
    except Exception as e:  # profiling is best-effort; the run must not break
        print(f"ntff shim unavailable: {e!r}")
        return False


# ---------------------------------------------------------------- entrypoint
def kernel(sh_vectors, cutoffs, receivers, inv_avg_num_neighbors) -> np.ndarray:
    global LAST_RESULTS
    from concourse.bass_utils import run_bass_kernel_spmd

    in_maps, caps, node_orders = shard_inputs(sh_vectors, cutoffs, receivers,
                                              inv_avg_num_neighbors)
    nc = build_nc(caps)

    trace = os.environ.get("KERNEL_TRACE", "0") == "1"
    if trace:
        trace = _install_ntff_shim()
    res = run_bass_kernel_spmd(nc, in_maps, core_ids=list(range(N_CORES)),
                               trace=trace)
    LAST_RESULTS = res

    full = np.empty((N_NODES, D_SH), dtype=np.float32)
    for c in range(N_CORES):
        o = res.results[c]["out"]                # row q -> node node_orders[q]
        blk = np.empty((NPAD, D_SH), dtype=np.float32)
        blk[node_orders[c]] = o
        full[c * NPC:(c + 1) * NPC] = blk[:NPC]
    return full
